# revision 13
# baseline (speedup 1.0000x reference)
"""MDCT (conv1d stride-512, kernel-1024, pad-512) as a Bass/Tile kernel on 8 trn2 cores.

Strategy
--------
out[b,k,j] = sum_t F[k,t] * xpad[b, j*512 + t],  x:[16,1,1048576] -> out:[16,512,2049]

* Data-parallel over batch: 2 batches per NeuronCore (8 cores).
* MDCT fold: the 2N=1024 window folds into an N=512 DCT-IV, halving matmul K:
    frame j window = [A_{j-1}, A_j]  (A_j = x[j*512:(j+1)*512])
    g2[u] = A[255-u] + A[256+u]  (u in [0,256), computed per block A)
    g1[v] = A[v]     - A[511-v]
    out[:,j] = -C'[:,0:256] @ g2(A_j) + C'[:,256:512] @ g1(A_{j-1})
  where C'[k,u] is extracted from the filter itself (least-squares over the two
  redundant copies of each coefficient present in F).
* On-chip: load x blocks in natural layout [block, sample] (2KB-contiguous DMA),
  fold on DVE (negative-stride reads), transpose [block,u]->[u,block] on the PE
  (identity transpose), matmul in float32r (TF32-like, 4x faster than fp32).
"""

import numpy as np

N = 512
B = 16
T = 2048
NCORES = 8
BPC = B // NCORES          # batches per core = 2
JCHUNK = 512               # frames per chunk (PSUM bank = 512 fp32)
NCHUNK = T // JCHUNK       # 4 full chunks; frame 2048 handled as tail
SAMP = N * T               # samples per batch

_compiled = None


def _build():
    import concourse.bass as bass
    import concourse.mybir as mybir
    from concourse import bacc
    from concourse.tile import TileContext
    from concourse.masks import make_identity

    f32 = mybir.dt.float32
    f32r = mybir.dt.float32r

    nc = bacc.Bacc("TRN2", target_bir_lowering=False, debug=False)

    xs_d = nc.dram_tensor("xs", [BPC, SAMP], f32, kind="ExternalInput").ap()
    w_d = nc.dram_tensor("wt", [4, 128, N], f32, kind="ExternalInput").ap()
    o_d = nc.dram_tensor("os", [BPC, N, T + 1], f32, kind="ExternalOutput").ap()

    with TileContext(nc) as tc:
        with tc.tile_pool(name="wp", bufs=1) as wp, \
             tc.tile_pool(name="xp", bufs=8) as xp, \
             tc.tile_pool(name="gp", bufs=8) as gp, \
             tc.tile_pool(name="mtp", bufs=2) as mtp, \
             tc.tile_pool(name="op", bufs=8) as op, \
             tc.tile_pool(name="tps", bufs=4, space="PSUM") as tps, \
             tc.tile_pool(name="ops", bufs=4, space="PSUM") as ops:

            ident = wp.tile([128, 128], f32, tag="ident")
            make_identity(nc, ident[:])
            z0 = wp.tile([128, 1], f32, tag="z0")
            nc.vector.memset(z0[:], 0.0)
            ident_r = wp.tile([128, 128], f32r, tag="identr")
            nc.vector.tensor_copy(out=ident_r[:], in_=ident[:])

            W = []
            for uc in range(4):
                w_t = wp.tile([128, N], f32r, tag=f"w{uc}")
                nc.gpsimd.dma_start(out=w_t[:], in_=w_d[uc])  # cast fp32 -> fp32r
                W.append(w_t)

            for b in range(BPC):
                prev_mt = None  # previous chunk's MT tiles (for col 0 = block j0-1)
                for jc in range(NCHUNK):
                    j0 = jc * JCHUNK
                    # ---- load 4 natural-layout tiles [128 blocks, 512 samples]
                    X = []
                    for t in range(4):
                        x_t = xp.tile([128, N], f32, tag="x")
                        s0 = (j0 + 128 * t) * N
                        nc.sync.dma_start(
                            out=x_t[:],
                            in_=xs_d[b, s0:s0 + 128 * N].rearrange("(p f) -> p f", p=128),
                        )
                        X.append(x_t)
                    # ---- fold on DVE -> G tiles [128 blocks, 512 u]
                    G = []
                    for t in range(4):
                        g_t = gp.tile([128, N], f32r, tag="g")
                        nc.vector.tensor_add(g_t[:, 0:256], X[t][:, 255::-1], X[t][:, 256:512])
                        nc.vector.tensor_sub(g_t[:, 256:512], X[t][:, 0:256], X[t][:, 511:255:-1])
                        G.append(g_t)
                    # ---- MT tiles [128 u, 513 blockcols] per u-chunk, fp32r
                    MT = [mtp.tile([128, JCHUNK + 1], f32r, tag=f"mt{uc}", name=f"mt{uc}")
                          for uc in range(4)]
                    for uc in (2, 3):
                        if jc == 0:
                            nc.vector.tensor_copy(out=MT[uc][:, 0:1], in_=z0[:])
                        else:
                            nc.vector.tensor_copy(out=MT[uc][:, 0:1], in_=prev_mt[uc][:, JCHUNK:JCHUNK + 1])
                    # ---- PE transposes [block,u] -> [u,block] + copies into MT
                    for t in range(4):
                        for uc in range(4):
                            p_t = tps.tile([128, 128], f32r, tag="tp")
                            nc.tensor.transpose(p_t[:], G[t][:, 128 * uc:128 * (uc + 1)], ident_r[:])
                            cp = nc.scalar.copy if uc < 2 else nc.vector.tensor_copy
                            cp(out=MT[uc][:, 1 + 128 * t:129 + 128 * t], in_=p_t[:])
                    # ---- main matmuls: psum[kc] = sum_uc W[uc][:,kc].T @ MT[uc]
                    for kc in range(4):
                        po = ops.tile([128, JCHUNK], mybir.dt.float32, tag="po")
                        for uc in range(4):
                            rhs = MT[uc][:, 1:JCHUNK + 1] if uc < 2 else MT[uc][:, 0:JCHUNK]
                            nc.tensor.matmul(
                                po[:], W[uc][:, 128 * kc:128 * (kc + 1)], rhs,
                                start=(uc == 0), stop=(uc == 3),
                            )
                        o_t = op.tile([128, JCHUNK], f32, tag="o")
                        nc.scalar.copy(out=o_t[:], in_=po[:])
                        nc.sync.dma_start(
                            out=o_d[b, 128 * kc:128 * (kc + 1), j0:j0 + JCHUNK],
                            in_=o_t[:],
                        )
                    prev_mt = MT

                # ---- tail frame j=2048: out[:,2048] = C'[:,256:] @ g1(A_2047)
                ot = op.tile([128, 4], f32, tag="otail")
                for kc in range(4):
                    pt = tps.tile([128, 4], mybir.dt.float32, tag="tp", name="pt")
                    for i, uc in enumerate((2, 3)):
                        nc.tensor.matmul(
                            pt[:, 0:1],
                            W[uc][:, 128 * kc:128 * (kc + 1)].bitcast(f32),
                            prev_mt[uc][:, JCHUNK:JCHUNK + 1].bitcast(f32),
                            start=(i == 0), stop=(i == 1),
                        )
                    nc.scalar.copy(out=ot[:, kc:kc + 1], in_=pt[:, 0:1])
                nc.sync.dma_start(
                    out=o_d[b, :, T:T + 1].rearrange("(c p) o -> p (c o)", p=128),
                    in_=ot[:],
                )

    nc.compile()
    return nc


def _weights(mdct_filter: np.ndarray) -> np.ndarray:
    """Extract DCT-IV weight tiles W[4,128,512] from the 1024-tap filter.

    Each C'[k,u] coefficient appears twice in F (up to sign); average the two
    copies (least squares) to minimize the fold residual.
    """
    F = mdct_filter.reshape(N, 2 * N).astype(np.float64)
    sideA = np.concatenate([-F[:, 768:1024], F[:, 0:256]], axis=1)
    sideB = -F[:, 767:255:-1]
    Cp = 0.5 * (sideA + sideB)  # [k, u]
    W = np.empty((4, 128, N), dtype=np.float32)
    W[0] = -Cp[:, 0:128].T
    W[1] = -Cp[:, 128:256].T
    W[2] = Cp[:, 256:384].T
    W[3] = Cp[:, 384:512].T
    return W


def kernel(x: np.ndarray, mdct_filter: np.ndarray, _trace=False) -> np.ndarray:
    global _compiled
    from concourse.bass_utils import run_bass_kernel_spmd

    if _compiled is None:
        _compiled = _build()
    nc = _compiled

    x = np.ascontiguousarray(np.asarray(x, dtype=np.float32)).reshape(B, SAMP)
    wt = _weights(np.asarray(mdct_filter, dtype=np.float32))

    in_maps = [
        {"xs": x[c * BPC:(c + 1) * BPC], "wt": wt}
        for c in range(NCORES)
    ]
    res = run_bass_kernel_spmd(nc, in_maps, core_ids=list(range(NCORES)),
                               trace=_trace)
    out = np.empty((B, N, T + 1), dtype=np.float32)
    for c in range(NCORES):
        out[c * BPC:(c + 1) * BPC] = res.results[c]["os"]
    if _trace:
        kernel._last_results = res
    return out
